# revision 21
# baseline (speedup 1.0000x reference)
"""Trainium2 Bass kernel: GQA causal self-attention with ALiBi.

Problem: B=4, T=2048, C=2048, 16 Q heads / 4 KV heads, head_dim=128, fp32.

Sharding (8 cores): DP2 x TP4. Core c = (bg, g) with bg = c//4 (batches
2bg, 2bg+1), g = c%4 (KV group g = Q heads 4g..4g+3 + KV head g). The
reference's ALiBi slope is constant within a KV group (slopes[h//4]), so
each core has a single slope. Host feeds x^T per batch (transpose-free
dataflow on chip) and sums the 4 partial Wo outputs per batch.

Numerics: logits are bounded above (~+6) so softmax runs without the
running-max pass. ALiBi decay truncates attention to a 1-prior-key-chunk
window (dropped keys have relative weight < e^-24).

Schedule: the per-block pipeline is [attention(g) | projections(g+1) |
out-proj(g)] so the PE never waits on the softmax or normalization
chains (keeps the tensor engine at its top p-state). Attention runs in
head pairs sharing one 2-bank PSUM tile per wave; PV lags S by 2 waves.
The ALiBi+causal mask is applied as a precomputed exp(bias) multiply in
bf16 (fast DVE mode) after the exp, and S/exp skip fully-masked query
columns. V is projected directly in [keys, head_dim] layout by using the
x chunk as the stationary operand (no PE transposes).
"""

import math
from contextlib import ExitStack

import ml_dtypes
import numpy as np

import concourse.bass as bass
import concourse.mybir as mybir
import concourse.tile as tile
from concourse import bacc
from concourse.bass_utils import run_bass_kernel_spmd


def _prefer_combined_act_tables():
    """Steer the act-table-load pass to a single set containing both exp
    and ln; otherwise it alternates between an exp-only and an ln-only
    set (1.3us reload per switch). Set ids must keep indexing the
    original act_info.json order (walrus remaps by that index), so
    instead of reordering, strip exp/ln from the sets that precede the
    combined one."""
    import concourse.hw_specs as hw_specs

    orig = hw_specs.get_activation_tables.__wrapped__

    def patched(arch):
        tabs = orig(arch)
        is_combined = {
            k: (
                any(str(f).lower().endswith("exp") for f in v)
                and any(str(f).lower().endswith("ln") for f in v)
            )
            for k, v in tabs.items()
        }
        if not any(is_combined.values()):
            return tabs
        out = {}
        seen_combined = False
        for k, v in tabs.items():
            if is_combined[k]:
                seen_combined = True
                out[k] = v
            elif not seen_combined:
                out[k] = {
                    f for f in v
                    if not str(f).lower().endswith(("exp", "ln"))
                }
            else:
                out[k] = v
        return out

    import functools
    cached = functools.cache(patched)
    hw_specs.get_activation_tables = cached
    bacc.get_activation_tables = cached


_prefer_combined_act_tables()

B, T, C = 4, 2048, 2048
HD = 128          # head dim
HPC = 4           # Q heads per core
QB = 512          # query block (attention tile free dim)
KC = 128          # key chunk
NQB = T // QB     # 4
NCC = C // 128    # 16 contraction chunks for projections
NG = 2 * NQB      # 8 block iterations per core (2 batches x 4 blocks)

F32 = mybir.dt.float32
F32R = mybir.dt.float32r
BF16 = mybir.dt.bfloat16
EXP = mybir.ActivationFunctionType.Exp
LN = mybir.ActivationFunctionType.Ln

_CACHE = {}


def build_kernel():
    nc = bacc.Bacc(
        "TRN2",
        target_bir_lowering=False,
        debug=False,
        enable_asserts=False,
        num_devices=8,
    )
    xT2 = nc.dram_tensor("xT2", [2, NQB, C, QB], BF16, kind="ExternalInput").ap()
    wq_d = nc.dram_tensor("wq", [C, HPC * HD], BF16, kind="ExternalInput").ap()
    wk_d = nc.dram_tensor("wk", [C, HD], BF16, kind="ExternalInput").ap()
    wv_d = nc.dram_tensor("wv", [C, HD], BF16, kind="ExternalInput").ap()
    wo_d = nc.dram_tensor("wo", [HPC * HD, C], BF16, kind="ExternalInput").ap()
    et_d = nc.dram_tensor("etiles", [5, KC, 2 * QB], BF16, kind="ExternalInput").ap()
    on_d = nc.dram_tensor("onesc", [128, 128], F32R, kind="ExternalInput").ap()
    ob_d = nc.dram_tensor("onesb", [128, 128], BF16, kind="ExternalInput").ap()
    id_d = nc.dram_tensor("ident", [128, 128], BF16, kind="ExternalInput").ap()
    outT = nc.dram_tensor("outT", [2, NQB, C, QB], F32, kind="ExternalOutput").ap()

    with ExitStack() as ctx:
        tc = ctx.enter_context(tile.TileContext(nc))
        ctx.enter_context(
            nc.allow_low_precision(reason="float32r is full fp32 width")
        )

        consts = ctx.enter_context(tc.tile_pool(name="consts", bufs=1))
        xpool = ctx.enter_context(tc.tile_pool(name="xpool", bufs=2))
        kvpool = ctx.enter_context(tc.tile_pool(name="kvpool", bufs=1))
        qpool = ctx.enter_context(tc.tile_pool(name="qpool", bufs=2))
        prp = ctx.enter_context(tc.tile_pool(name="prp", bufs=2))
        ptp = ctx.enter_context(tc.tile_pool(name="ptp", bufs=3))
        recp = ctx.enter_context(tc.tile_pool(name="recp", bufs=4))
        bcp = ctx.enter_context(tc.tile_pool(name="bcp", bufs=4))
        yp = ctx.enter_context(tc.tile_pool(name="yp", bufs=2))
        op = ctx.enter_context(tc.tile_pool(name="op", bufs=2))

        # PSUM: s(1x2) + y(1x2) + dn(2x1) + pp(2x1) = 8 banks
        ps = ctx.enter_context(tc.tile_pool(name="ps", bufs=2, space="PSUM"))

        # resident weights / constants. wk first (K proj is the first PE
        # work); bulky wq/wo go on other engines' DMA queues so the x
        # strips aren't stuck behind them on the Sync queue.
        wk_sb = consts.tile([128, NCC, HD], BF16)
        nc.sync.dma_start(wk_sb, wk_d.rearrange("(cc p) d -> p cc d", p=128))
        wv_sb = consts.tile([128, NCC, HD], BF16)
        nc.sync.dma_start(wv_sb, wv_d.rearrange("(cc p) d -> p cc d", p=128))
        wq_sb = consts.tile([128, NCC, HPC * HD], BF16)
        nc.gpsimd.dma_start(wq_sb, wq_d.rearrange("(cc p) d -> p cc d", p=128))
        e_sb = consts.tile([128, 5, 2, QB], BF16)
        nc.gpsimd.dma_start(
            e_sb, et_d.rearrange("m p (two f) -> p m two f", two=2)
        )
        ones = consts.tile([128, 128], F32R)
        nc.gpsimd.dma_start(ones, on_d)
        ones_bf = consts.tile([128, 128], BF16)
        nc.gpsimd.dma_start(ones_bf, ob_d)
        ident = consts.tile([128, 128], BF16)
        nc.gpsimd.dma_start(ident, id_d)
        wo_sb = consts.tile([128, HPC, C], BF16)
        nc.scalar.dma_start(wo_sb, wo_d.rearrange("(hc p) c -> p hc c", p=128))

        # K/V for the current block + the tail chunk of the previous block
        # (the ALiBi window never reaches further back).
        kt_cur = kvpool.tile([128, QB], BF16, tag="ktc")
        kt_prev = kvpool.tile([128, KC], BF16, tag="ktp")
        v_cur = kvpool.tile([128, 4, HD], BF16, tag="vc")
        v_prev = kvpool.tile([128, HD], BF16, tag="vp")

        # pT_raw slots hold stale data in masked columns across reuse; a
        # one-time zero fill guarantees those columns are finite (they are
        # multiplied by an exact 0 in the mask tile before use).
        for _ in range(2):
            pr0 = prp.tile([128, 2, QB], BF16, tag="pr")
            nc.vector.memset(pr0, 0.0)

        strips = {}
        qTs = {}

        def dma_strip(g):
            b, tb = divmod(g, 4)
            xt = xpool.tile([128, NCC, QB], BF16, tag="x")
            src = xT2[b, tb].rearrange("(cc p) f -> p cc f", p=128)
            for q in range(0, NCC, 4):
                nc.sync.dma_start(xt[:, q:q + 4, :], src[:, q:q + 4, :])
            strips[g] = xt

        def emit_proj(g, pre_fill=None, mid_fill=None):
            """K/V/Q projections for block g. pre/mid fillers are PE work
            (previous pair's norm matmuls) slotted where their inputs are
            ready without stalling the PE."""
            tb = g % 4
            xts = strips.pop(g)
            # K projection -> [HD, keys] (natural layout for S)
            ps_k = ps.tile([128, QB], F32, tag="pp")
            for cc in range(NCC):
                nc.tensor.matmul(
                    ps_k, lhsT=wk_sb[:, cc, :], rhs=xts[:, cc, :],
                    start=(cc == 0), stop=(cc == NCC - 1),
                )
            if tb > 0:
                nc.scalar.copy(kt_prev, kt_cur[:, 3 * KC:4 * KC])
            nc.scalar.copy(kt_cur, ps_k)
            if pre_fill is not None:
                pre_fill()
            # V projection [HD, keys], then transpose chunks to [keys, HD]
            ps_v = ps.tile([128, QB], F32, tag="pp")
            for cc in range(NCC):
                nc.tensor.matmul(
                    ps_v, lhsT=wv_sb[:, cc, :], rhs=xts[:, cc, :],
                    start=(cc == 0), stop=(cc == NCC - 1),
                )
            vT_sb = prp.tile([128, QB], BF16, tag="vt")
            nc.scalar.copy(vT_sb, ps_v)
            if tb > 0:
                nc.vector.tensor_copy(v_prev, v_cur[:, 3, :])
            for kc in range(4):
                tp = ps.tile([128, KC], BF16, tag="pp")
                nc.tensor.transpose(tp, vT_sb[:, kc * KC:(kc + 1) * KC], ident)
                nc.vector.tensor_copy(v_cur[:, kc, :], tp)
            if mid_fill is not None:
                mid_fill()
            # Q projection
            qT = qpool.tile([128, HPC, QB], BF16, tag="q")
            for h in range(HPC):
                ps_q = ps.tile([128, QB], F32, tag="pp")
                for cc in range(NCC):
                    nc.tensor.matmul(
                        ps_q,
                        lhsT=wq_sb[:, cc, h * HD:(h + 1) * HD],
                        rhs=xts[:, cc, :],
                        start=(cc == 0), stop=(cc == NCC - 1),
                    )
                nc.scalar.copy(qT[:, h, :], ps_q)
            qTs[g] = qT

        def emit_attention(g, pair, fillers):
            """One head pair's attention for block g. fillers: wave -> fn
            emitting the previous pair's norm matmuls as PE filler."""
            tb = g % 4
            qT = qTs.pop(g) if pair == 1 else qTs[g]
            ms = list(range(5)) if tb > 0 else list(range(1, 5))
            L = len(ms)
            D = 2  # PV lags S by D waves
            y_pair = ps.tile([128, 2, QB], F32, tag="y", bufs=1)
            # den broadcast to all partitions via a full all-ones stationary
            # (1-output-partition matmuls run at half rate; this also kills
            # the separate broadcast matmul)
            dns = [
                ps.tile([128, QB], F32, tag="dn", bufs=2, name=f"dn{pair}{hi}")
                for hi in range(2)
            ]
            pts = {}
            for i in range(L + D):
                if i < L:
                    m = ms[i]
                    s0 = max(0, (m - 1) * KC)
                    s_pair = ps.tile([128, 2, QB], F32, tag="s", bufs=1)
                    for hi in range(2):
                        h = 2 * pair + hi
                        lhsT = (
                            kt_prev if m == 0
                            else kt_cur[:, (m - 1) * KC:m * KC]
                        )
                        nc.tensor.matmul(
                            s_pair[:, hi, s0:QB],
                            lhsT=lhsT,
                            rhs=qT[:, h, s0:QB],
                        )
                    pr = prp.tile([128, 2, QB], BF16, tag="pr")
                    nc.scalar.activation(
                        pr[:, :, s0:QB], s_pair[:, :, s0:QB], EXP
                    )
                    pt = ptp.tile([128, 2, QB], BF16, tag="pt")
                    nc.vector.tensor_mul(pt, pr, e_sb[:, m])
                    pts[i] = pt
                j = i - D
                if 0 <= j < L:
                    mj = ms[j]
                    t0 = max(0, (mj - 1) * KC)
                    pt = pts.pop(j)
                    v_sl = v_prev if mj == 0 else v_cur[:, mj - 1, :]
                    for hi in range(2):
                        nc.tensor.matmul(
                            y_pair[:, hi, t0:QB],
                            lhsT=v_sl,
                            rhs=pt[:, hi, t0:QB],
                            start=(j == 0),
                            stop=(j == L - 1),
                            skip_group_check=True,
                        )
                    # softmax denominator, broadcast to all partitions:
                    # colsum of pt via all-ones stationary, accumulated on PE
                    for hi in range(2):
                        nc.tensor.matmul(
                            dns[hi][:, t0:QB],
                            lhsT=ones_bf,
                            rhs=pt[:, hi, t0:QB],
                            start=(j == 0),
                            stop=(j == L - 1),
                            skip_group_check=True,
                        )
                fill = fillers.get(i)
                if fill is not None:
                    fill()
            return y_pair, dns

        def make_norm(pair, dns, y_pair, y_sb_t):
            """Softmax normalization: 1/den as exp(-ln(den)) on ACT (the
            DVE reciprocal is ~6.5ns/element at any width), then scale on
            DVE. den arrives already broadcast across partitions."""
            recs = {}

            def ln_step():
                # ACT only: runs right after the pair's last colsum
                for hi in range(2):
                    lns = recp.tile([128, QB], F32R, tag="lns", bufs=2)
                    nc.scalar.activation(lns, dns[hi], LN)
                    rec = recp.tile([128, QB], F32R, tag="rec")
                    nc.scalar.activation(rec, lns, EXP, scale=-1.0)
                    recs[hi] = rec

            def bc_step():
                for hi in range(2):
                    nc.vector.tensor_mul(
                        y_sb_t[:, 2 * pair + hi, :], y_pair[:, hi, :],
                        recs[hi],
                    )

            return ln_step, bc_step

        outT_r = [
            [
                outT[b, tb].rearrange("(cc p) f -> p cc f", p=128)
                for tb in range(NQB)
            ]
            for b in range(2)
        ]

        def emit_oproj(g, y_sb_t):
            b, tb = divmod(g, 4)
            o_sb = None
            for co in range(16):
                o_ps = ps.tile([128, QB], F32, tag="pp")
                for hc in range(HPC):
                    nc.tensor.matmul(
                        o_ps,
                        lhsT=wo_sb[:, hc, co * 128:(co + 1) * 128],
                        rhs=y_sb_t[:, hc, :],
                        start=(hc == 0), stop=(hc == HPC - 1),
                    )
                if co % 4 == 0:
                    o_sb = op.tile([128, 4, QB], F32, tag="o")
                if co % 2 == 0:
                    nc.vector.tensor_copy(o_sb[:, co % 4, :], o_ps)
                else:
                    nc.scalar.copy(o_sb[:, co % 4, :], o_ps)
                if co % 4 == 3:
                    nc.sync.dma_start(
                        outT_r[b][tb][:, co - 3:co + 1, :], o_sb
                    )

        dma_strip(0)
        dma_strip(1)
        emit_proj(0)
        for g in range(NG):
            y_sb_t = yp.tile([128, HPC, QB], BF16, tag="ysb")
            y0, dns0 = emit_attention(g, 0, fillers={})
            ln0, bc0 = make_norm(0, dns0, y0, y_sb_t)
            ln0()
            y1, dns1 = emit_attention(g, 1, fillers={1: bc0})
            ln1, bc1 = make_norm(1, dns1, y1, y_sb_t)
            ln1()
            if g + 2 < NG:
                dma_strip(g + 2)
            if g + 1 < NG:
                emit_proj(g + 1, pre_fill=bc1)
            else:
                bc1()
            emit_oproj(g, y_sb_t)

    nc.compile()
    return nc


def make_etiles(sigma):
    """E[m][p,f] = exp(sigma*((m-1)*128 + p - f)) with the causal mask as
    exact zeros; duplicated along the free axis for head-pair tiles. m=0
    is the unmasked prior chunk, m=1..4 the diagonal chunks."""
    p = np.arange(KC, dtype=np.float32)[:, None]
    f = np.arange(QB, dtype=np.float32)[None, :]
    out = np.zeros((5, KC, QB), np.float32)
    for m in range(5):
        o = (m - 1) * 128
        d = o + p - f
        valid = p <= f - o
        out[m] = np.where(valid, np.exp(sigma * np.minimum(d, 0.0)), 0.0)
    out2 = np.repeat(out[:, :, None, :], 2, axis=2).reshape(5, KC, 2 * QB)
    return out2.astype(ml_dtypes.bfloat16)


def kernel(x, Wq, Wk, Wv, Wo):
    import os
    import time

    dbg = os.environ.get("KERNEL_DEBUG") == "1"
    t0 = time.time()

    def tick(msg):
        nonlocal t0
        if dbg:
            print(f"[kernel] {msg}: {time.time() - t0:.2f}s", flush=True)
        t0 = time.time()

    x = np.ascontiguousarray(np.asarray(x, np.float32))
    Wq = np.ascontiguousarray(np.asarray(Wq, np.float32))
    Wk = np.ascontiguousarray(np.asarray(Wk, np.float32))
    Wv = np.ascontiguousarray(np.asarray(Wv, np.float32))
    Wo = np.ascontiguousarray(np.asarray(Wo, np.float32))

    tick("input prep")
    if "nc" not in _CACHE:
        _CACHE["nc"] = build_kernel()
        tick("build_kernel")
    nc = _CACHE["nc"]

    s = 1.0 / math.sqrt(HD)
    slopes = [2.0 ** -0.5, 0.5, 2.0 ** -1.5, 0.25]
    BF = ml_dtypes.bfloat16

    in_maps = []
    for c in range(8):
        bg, g = c // 4, c % 4
        xT2 = np.stack([
            np.ascontiguousarray(
                x[2 * bg + i].T.reshape(C, NQB, QB).transpose(1, 0, 2)
            )
            for i in range(2)
        ])
        in_maps.append({
            "xT2": xT2.astype(BF),
            "wq": (Wq[:, g * 512:(g + 1) * 512] * s).astype(BF),
            "wk": Wk[:, g * HD:(g + 1) * HD].astype(BF),
            "wv": Wv[:, g * HD:(g + 1) * HD].astype(BF),
            "wo": Wo[g * 512:(g + 1) * 512, :].astype(BF),
            "etiles": make_etiles(slopes[g]),
            "onesc": np.ones((128, 128), np.float32),
            "onesb": np.ones((128, 128), BF),
            "ident": np.eye(128, dtype=BF),
        })

    tick("in_maps prep")
    res = run_bass_kernel_spmd(nc, in_maps, core_ids=list(range(8)))
    tick("device run")
    out = np.zeros((B, T, C), np.float32)
    for c in range(8):
        bg, g = c // 4, c % 4
        oT = res.results[c]["outT"]
        for i in range(2):
            out[2 * bg + i] += oT[i].transpose(0, 2, 1).reshape(T, C)
    tick("gather")
    return out
